# revision 4
# baseline (speedup 1.0000x reference)
"""Trainium2 Bass kernel for LNLinear + KillingRelu + KillingMaxPool.

Math (per batch b, channels f, sl3-coords k, positions n):
  x1 = W_lin @ x                      (channel mix, K=128)
  d  = W_relu @ x1 = (W_relu@W_lin)@x (fused on host -> K=128)
  kf = sum_kl x1_k K6[k,l] d_l        (Killing form, K6 = 6*G)
  x2 = x1 + relu(kf)*d
  d2 = W_pool @ x2                    (K=256)
  kf2 = Killing(x2, d2);  out = x2[:, :, argmax_n kf2]

K6 structure: (K6 v)_l = 6*v_perm(l) for l in 0..5 with perm=(2,4,0,5,1,3),
(K6 v)_6 = 6*(2 v6 - v7), (K6 v)_7 = 6*(2 v7 - v6).  The scale 6 commutes
with relu and argmax, so we compute the unscaled kfu and use relu(6*kfu).

Sharding: batch b -> core b (8 cores), weights replicated. Device outputs
x2 (full) + per-f argmax index; host performs the final index gather.
"""

import numpy as np

import concourse.bacc as bacc
import concourse.bass as bass
import concourse.mybir as mybir
import concourse.tile as tile
from concourse.bass_utils import run_bass_kernel_spmd

B, CIN, COUT, KD, N = 8, 128, 256, 8, 4096
NCHUNK = 256
NCH = N // NCHUNK
F32 = mybir.dt.float32
F32R = mybir.dt.float32r
BF16 = mybir.dt.bfloat16
PERM = (2, 4, 0, 5, 1, 3)  # (K6 v)_l = 6 * v_PERM[l]  for l in 0..5


def _mm(nc, out_ap, lhsT_ap, rhs_ap, start=True, stop=True):
    nc.tensor.matmul(out_ap, lhsT_ap, rhs_ap, start=start, stop=stop)


def build_program(repeat=1):
    nc = bacc.Bacc("TRN2", target_bir_lowering=False, debug=False)

    x_in = nc.dram_tensor("x", [CIN, KD, N], F32R, kind="ExternalInput")
    wlin = nc.dram_tensor("wlin", [CIN, COUT], F32R, kind="ExternalInput")
    wrl = nc.dram_tensor("wrl", [CIN, COUT], F32R, kind="ExternalInput")
    # wpool layout [g, kh, f]: WpoolT[g, kh, f] = W_pool[f, kh*128+g]
    wpool = nc.dram_tensor("wpool", [128, 2, COUT], F32R, kind="ExternalInput")

    x2_out = nc.dram_tensor("x2o", [COUT, KD, N], F32R, kind="ExternalOutput")
    idx_out = nc.dram_tensor("idxo", [COUT, 8], mybir.dt.uint32, kind="ExternalOutput")

    AL = mybir.AluOpType

    with tile.TileContext(nc) as tc:
        with (
            tc.tile_pool(name="wpool_p", bufs=1) as wp,
            tc.tile_pool(name="xc", bufs=2) as xcp,
            tc.tile_pool(name="s1", bufs=2) as s1p,
            tc.tile_pool(name="tmp", bufs=1) as tmp,
            tc.tile_pool(name="x2p", bufs=2) as x2p,
            tc.tile_pool(name="kf2", bufs=1) as kf2p,
            tc.tile_pool(name="ps", bufs=2, space="PSUM") as psp,
            tc.tile_pool(name="outp", bufs=1) as outp,
        ):
            # --- weights resident in SBUF ---
            wlin_sb = wp.tile([CIN, COUT], F32R, tag="wlin")
            wrl_sb = wp.tile([CIN, COUT], F32R, tag="wrl")
            wpool_sb = wp.tile([128, 2, COUT], F32R, tag="wpool")
            nc.sync.dma_start(out=wlin_sb[:], in_=wlin[:])
            nc.sync.dma_start(out=wrl_sb[:], in_=wrl[:])
            nc.sync.dma_start(out=wpool_sb[:], in_=wpool[:])

            # kf2u planes persist across chunks (argmax input), per f-half
            kf2_pl = [kf2p.tile([128, N], F32, tag=f"kf2_{fh}", name=f"kf2pl{fh}") for fh in (0, 1)]

            if repeat != 1:
                rep_ctx = tc.For_i(0, repeat)
                rep_ctx.__enter__()

            for c in range(NCH):
                n0 = c * NCHUNK
                xc = xcp.tile([CIN, KD, NCHUNK], F32R, tag="xc")
                nc.sync.dma_start(out=xc[:], in_=x_in[:, :, n0 : n0 + NCHUNK])
                xc2d = xc.rearrange("p k n -> p (k n)")

                x2_sb = []
                for fh in (0, 1):
                    # ---- x1 = W_lin @ x ----
                    x1ps = psp.tile([128, KD * NCHUNK], F32, tag="ps")
                    for j in range(0, KD * NCHUNK, 512):
                        _mm(
                            nc,
                            x1ps[:, j : j + 512],
                            wlin_sb[:, fh * 128 : fh * 128 + 128],
                            xc2d[:, j : j + 512],
                        )
                    x1 = s1p.tile([128, KD, NCHUNK], F32, tag="x1")
                    nc.scalar.copy(x1.rearrange("p k n -> p (k n)")[:], x1ps[:])

                    # ---- d = (W_relu W_lin) @ x ----
                    dps = psp.tile([128, KD * NCHUNK], F32, tag="ps")
                    for j in range(0, KD * NCHUNK, 512):
                        _mm(
                            nc,
                            dps[:, j : j + 512],
                            wrl_sb[:, fh * 128 : fh * 128 + 128],
                            xc2d[:, j : j + 512],
                        )
                    d = s1p.tile([128, KD, NCHUNK], F32, tag="d")
                    nc.scalar.copy(d.rearrange("p k n -> p (k n)")[:], dps[:])

                    # ---- Killing form kfu = sum_l x1perm_l * d_l ----
                    u = tmp.tile([128, 2, NCHUNK], F32, tag="u")
                    nc.vector.scalar_tensor_tensor(
                        out=u[:, 0, :], in0=x1[:, 6, :], scalar=2.0,
                        in1=x1[:, 7, :], op0=AL.mult, op1=AL.subtract,
                    )
                    nc.vector.scalar_tensor_tensor(
                        out=u[:, 1, :], in0=x1[:, 7, :], scalar=2.0,
                        in1=x1[:, 6, :], op0=AL.mult, op1=AL.subtract,
                    )
                    p = tmp.tile([128, KD, NCHUNK], F32, tag="p")
                    for l in range(6):
                        nc.vector.tensor_tensor(
                            out=p[:, l, :], in0=x1[:, PERM[l], :], in1=d[:, l, :],
                            op=AL.mult,
                        )
                    for l in (6, 7):
                        nc.vector.tensor_tensor(
                            out=p[:, l, :], in0=u[:, l - 6, :], in1=d[:, l, :],
                            op=AL.mult,
                        )
                    t1 = tmp.tile([128, 4, NCHUNK], F32, tag="t1")
                    nc.vector.tensor_tensor(
                        out=t1[:], in0=p[:, 0:4, :], in1=p[:, 4:8, :], op=AL.add
                    )
                    t2 = tmp.tile([128, 2, NCHUNK], F32, tag="t2")
                    nc.vector.tensor_tensor(
                        out=t2[:], in0=t1[:, 0:2, :], in1=t1[:, 2:4, :], op=AL.add
                    )
                    kfu = tmp.tile([128, NCHUNK], F32, tag="kfu")
                    nc.vector.tensor_tensor(
                        out=kfu[:], in0=t2[:, 0, :], in1=t2[:, 1, :], op=AL.add
                    )
                    # r = relu(6*kfu)
                    r = tmp.tile([128, NCHUNK], F32, tag="r")
                    nc.scalar.activation(
                        r[:], kfu[:], mybir.ActivationFunctionType.Relu, scale=6.0
                    )

                    # ---- x2 = x1 + r*d ----
                    q = tmp.tile([128, KD, NCHUNK], F32, tag="q")
                    for l in range(KD):
                        nc.vector.tensor_tensor(
                            out=q[:, l, :], in0=d[:, l, :], in1=r[:], op=AL.mult
                        )
                    x2 = x2p.tile([128, KD, NCHUNK], F32R, tag=f"x2_{fh}")
                    nc.vector.tensor_tensor(
                        out=x2.rearrange("p k n -> p (k n)")[:],
                        in0=x1.rearrange("p k n -> p (k n)")[:],
                        in1=q.rearrange("p k n -> p (k n)")[:],
                        op=AL.add,
                    )
                    x2_sb.append(x2)
                    nc.sync.dma_start(
                        out=x2_out[fh * 128 : fh * 128 + 128, :, n0 : n0 + NCHUNK],
                        in_=x2[:],
                    )

                for fh in (0, 1):
                    # ---- d2 = W_pool @ x2  (K=256) ----
                    d2ps = psp.tile([128, KD * NCHUNK], F32, tag="ps")
                    for j in range(0, KD * NCHUNK, 512):
                        _mm(
                            nc,
                            d2ps[:, j : j + 512],
                            wpool_sb[:, 0, fh * 128 : fh * 128 + 128],
                            x2_sb[0].rearrange("p k n -> p (k n)")[:, j : j + 512],
                            start=True, stop=False,
                        )
                        _mm(
                            nc,
                            d2ps[:, j : j + 512],
                            wpool_sb[:, 1, fh * 128 : fh * 128 + 128],
                            x2_sb[1].rearrange("p k n -> p (k n)")[:, j : j + 512],
                            start=False, stop=True,
                        )
                    d2 = tmp.tile([128, KD, NCHUNK], F32, tag="d2")
                    nc.scalar.copy(d2.rearrange("p k n -> p (k n)")[:], d2ps[:])

                    x2 = x2_sb[fh]
                    u2 = tmp.tile([128, 2, NCHUNK], F32, tag="u2")
                    nc.vector.scalar_tensor_tensor(
                        out=u2[:, 0, :], in0=x2[:, 6, :], scalar=2.0,
                        in1=x2[:, 7, :], op0=AL.mult, op1=AL.subtract,
                    )
                    nc.vector.scalar_tensor_tensor(
                        out=u2[:, 1, :], in0=x2[:, 7, :], scalar=2.0,
                        in1=x2[:, 6, :], op0=AL.mult, op1=AL.subtract,
                    )
                    p2 = tmp.tile([128, KD, NCHUNK], F32, tag="p2")
                    for l in range(6):
                        nc.vector.tensor_tensor(
                            out=p2[:, l, :], in0=x2[:, PERM[l], :], in1=d2[:, l, :],
                            op=AL.mult,
                        )
                    for l in (6, 7):
                        nc.vector.tensor_tensor(
                            out=p2[:, l, :], in0=u2[:, l - 6, :], in1=d2[:, l, :],
                            op=AL.mult,
                        )
                    s1 = tmp.tile([128, 4, NCHUNK], F32, tag="s1t")
                    nc.vector.tensor_tensor(
                        out=s1[:], in0=p2[:, 0:4, :], in1=p2[:, 4:8, :], op=AL.add
                    )
                    s2 = tmp.tile([128, 2, NCHUNK], F32, tag="s2t")
                    nc.vector.tensor_tensor(
                        out=s2[:], in0=s1[:, 0:2, :], in1=s1[:, 2:4, :], op=AL.add
                    )
                    nc.vector.tensor_tensor(
                        out=kf2_pl[fh][:, n0 : n0 + NCHUNK],
                        in0=s2[:, 0, :], in1=s2[:, 1, :], op=AL.add,
                    )

            # ---- argmax over N per (f) ----
            for fh in (0, 1):
                mx = outp.tile([128, 8], F32, tag=f"mx_{fh}")
                nc.vector.max(mx[:], kf2_pl[fh][:])
                ix = outp.tile([128, 8], mybir.dt.uint32, tag=f"ix_{fh}")
                nc.vector.max_index(ix[:], mx[:], kf2_pl[fh][:])
                nc.sync.dma_start(
                    out=idx_out[fh * 128 : fh * 128 + 128, :], in_=ix[:]
                )

            if repeat != 1:
                rep_ctx.__exit__(None, None, None)

    nc.compile()
    return nc


_NC_CACHE = None
LAST_RESULTS = None


def kernel(x, W_lin, W_relu, W_pool):
    global _NC_CACHE
    if _NC_CACHE is None:
        _NC_CACHE = build_program()
    nc = _NC_CACHE

    wlin_t = np.ascontiguousarray(W_lin.T.astype(np.float32))            # [128, 256]
    wrl_t = np.ascontiguousarray((W_relu @ W_lin).T.astype(np.float32))  # [128, 256]
    # [g, kh, f] = W_pool[f, kh*128+g]
    wp = np.ascontiguousarray(
        W_pool.astype(np.float32).reshape(COUT, 2, 128).transpose(2, 1, 0)
    )

    in_maps = [
        {
            "x": np.ascontiguousarray(x[b].astype(np.float32)),
            "wlin": wlin_t,
            "wrl": wrl_t,
            "wpool": wp,
        }
        for b in range(B)
    ]
    import os
    res = run_bass_kernel_spmd(
        nc, in_maps, list(range(B)), trace=bool(os.environ.get("KTRACE"))
    )
    global LAST_RESULTS
    LAST_RESULTS = res

    # Killing metric (fp64) for the host-side top-8 rescore
    G = np.zeros((8, 8), np.float64)
    for a, bb in [(0, 2), (1, 4), (3, 5)]:
        G[a, bb] = G[bb, a] = 1.0
    G[6, 6] = G[7, 7] = 2.0
    G[6, 7] = G[7, 6] = -1.0
    K6 = 6.0 * G
    Wp64 = W_pool.astype(np.float64)

    out = np.empty((B, COUT, KD), np.float32)
    ar = np.arange(COUT)
    for b in range(B):
        x2 = res.results[b]["x2o"]                    # [256, 8, 4096]
        cand = res.results[b]["idxo"].astype(np.int64)  # [256, 8] top-8 indices
        # exact rescore of the device's top-8 candidates (guards near-ties
        # against reduced-precision matmul rounding on device)
        xsel = x2[:, :, cand].astype(np.float64)      # [g, k, f, j]
        df = np.einsum("fg,gkfj->fkj", Wp64, xsel)    # d2 at candidate cols
        xf = x2[ar[:, None], :, cand].astype(np.float64)  # [f, j, k]
        kf2c = np.einsum("fjk,kl,flj->fj", xf, K6, df)
        jbest = kf2c.argmax(1)
        idx = cand[ar, jbest]
        out[b] = x2[ar, :, idx]
    return out

